# revision 23
# baseline (speedup 1.0000x reference)
"""Distributed Trainium2 kernel for a pre-norm transformer block (BasicFormerBlock).

Sharding: sequence-parallel over 8 NeuronCores. Core i owns sequence blocks
{i, 15-i} (2 x 128 tokens x 4 batches = 1024 rows). LN/QKV/attention-queries/
Wo/FFN are all local; the only collectives are AllGathers of K and V in fp8,
chunked per batch-pair (K01,V01,K23,V23) so attention on early batches
overlaps the later gathers. Causal attention is load-balanced exactly: every
core's two query blocks cover 17 kv-tiles of score work. The schedule is
core-independent (one SPMD graph); per-core causal masks are input data.

Compute dtype: bf16 on the TensorEngine (fp8 K/V), fp32 stats/residuals/
accumulation. Attention softmax: per-head scores in two [P,1536] fp32 PSUM
chunks -> merged Exp ACTIVATEs -> bf16 probs; denominators batch-reciprocal'd
(one DVE reciprocal per batch instead of per head).
"""

import sys
import numpy as np

for _p in ("/opt/trn_rl_repo", "/root/.axon_site/_ro/trn_rl_repo"):
    if _p not in sys.path:
        sys.path.append(_p)

import ml_dtypes
import concourse.bass as bass
import concourse.tile as tile
from concourse import mybir
from concourse.bass_utils import run_bass_kernel_spmd
from concourse.masks import make_identity
from concourse.vector_clock import ScopedClock


class PatchedBass(bass.Bass):
    """The staged walrus build rejects sem-eq waits on InstDrain (the new
    butterfly barrier) and allows at most one sync wait per CTRL instruction.
    Emit the legacy PSEUDO_SYNC_BARRIER (NRT expands it at load time)."""

    def multi_engine_barrier(self, engines):
        if set(engines) == set(self.engines):
            self._nrt_pseudo_barrier()
        else:
            super().multi_engine_barrier(engines)


class PatchedTC(tile.TileContext):
    MAXW = 1  # walrus CTRL instructions accept one sync wait

    def _drain_and_barrier(self, tick_clock, wait_clock):
        drain_inst = self.nc.sync.drain()
        wait_clock.add_sem_waits(
            drain_inst.ins, ScopedClock({None: tick_clock.global_clock}))
        si = drain_inst.ins.sync_info
        waits = list(si.on_wait or []) if si else []
        if len(waits) > self.MAXW:
            si.on_wait = waits[:self.MAXW]
            for i in range(self.MAXW, len(waits), self.MAXW):
                nop = self.nc.sync.nop(nofuse=True, hint=f"drainwait{i}")
                nop.ins.sync_info = mybir.SyncInfo(
                    on_wait=waits[i:i + self.MAXW], on_update=[])
        self.nc.all_engine_barrier()
        popped = self.nc._tile_sem_poison_stack.pop()
        assert popped is self._sem_poison
        self.nc.clear_and_free_semaphores(list(self.sems.allocated().values()))
        self.nc.all_engine_barrier()

BF16 = mybir.dt.bfloat16
F32 = mybir.dt.float32
FP8 = mybir.dt.float8e4
NPBF16 = ml_dtypes.bfloat16

H = 16
B = 4
S = 2048
D = 1024
F = 4096
P = 128
NC = 8
NBLK = S // P          # 16 seq blocks
SCALE = (1024.0 / 16.0) ** 0.5
EPS = 1e-12
EXP_OFF = -15.0        # constant subtracted inside exp; cancels in softmax

# kv slot s -> (source rank, j) in the chunked AllGather buffers.
# Slots 0..7 hold seq blocks 0..7 (rank r's j=0 block is block r).
# Slots 8+t hold seq block 15-t (rank t's j=1 block) -- host flips mp2.
def kv_src(s):
    return (s, 0) if s < 8 else (s - 8, 1)


def build_graph(vb_nonzero: bool, b2_nonzero: bool):
    nc = PatchedBass()

    x_ext = nc.declare_dram_parameter("x", [8, P, D], F32, isOutput=False)
    wq_ext = nc.declare_dram_parameter("wq", [P, 8, 8, P], BF16, isOutput=False)
    wk_ext = nc.declare_dram_parameter("wk", [P, 8, 8, P], BF16, isOutput=False)
    wv_ext = nc.declare_dram_parameter("wv", [P, 8, D], BF16, isOutput=False)
    wo_ext = nc.declare_dram_parameter("wo", [P, 8, D], BF16, isOutput=False)
    w1_ext = nc.declare_dram_parameter("w1", [P, 8, 32, P], BF16, isOutput=False)
    w2_ext = nc.declare_dram_parameter("w2", [P, 32, D], BF16, isOutput=False)
    qb_ext = nc.declare_dram_parameter("qb", [P, 8], F32, isOutput=False)
    kb_ext = nc.declare_dram_parameter("kb", [P, 8], F32, isOutput=False)
    vb_ext = nc.declare_dram_parameter("vb", [P, 8], F32, isOutput=False)
    y1b_ext = nc.declare_dram_parameter("y1b", [P, 32], F32, isOutput=False)
    b2_ext = nc.declare_dram_parameter("b2t", [P, D], F32, isOutput=False)
    mp1_ext = nc.declare_dram_parameter("mp1", [P, 8, P], BF16, isOutput=False)
    mp2_ext = nc.declare_dram_parameter("mp2", [P, 8, P], BF16, isOutput=False)
    out_ext = nc.declare_dram_parameter("out", [8, P, D], F32, isOutput=True)

    with PatchedTC(nc) as tc:
        _build_tile(nc, tc, locals(), vb_nonzero, b2_nonzero)
    _elide_pe_incs(nc)
    _split_sync_waits(nc)
    return nc


def _elide_pe_incs(nc):
    """Every PE matmul carries a +1 semaphore increment (a serialized
    ~26ns EVT_SEM register write).  Only increments some wait actually
    references are needed; PE instructions complete in program order, so
    dropping unwaited increments and renumbering thresholds is exact."""
    from collections import defaultdict
    incs = defaultdict(list)    # sem id -> [(inst, update)]
    waits = defaultdict(list)   # sem id -> [wait]
    eng_of = {}
    ok = defaultdict(lambda: True)
    for fn in nc.m.functions:
        for blk in fn.blocks:
            for inst in blk.instructions:
                si = inst.sync_info
                if not si:
                    continue
                for u in (si.on_update or []):
                    incs[u.id].append((inst, u))
                    if u.update_mode != 'sem-inc' or u.update_value != 1:
                        ok[u.id] = False
                    if u.id in eng_of and eng_of[u.id] != inst.engine:
                        ok[u.id] = False
                    eng_of[u.id] = inst.engine
                for w in (si.on_wait or []):
                    waits[w.id].append(w)
                    if w.wait_mode != 'sem-ge-imm' or w.wait_reg is not None:
                        ok[w.id] = False
    for sid, lst in incs.items():
        if not ok[sid] or str(eng_of.get(sid)) != 'EngineType.PE':
            continue
        wl = waits.get(sid, [])
        needed = sorted({w.wait_value for w in wl if w.wait_value and w.wait_value > 0})
        if not needed or len(needed) >= len(lst):
            continue
        needed_set = set(needed)
        # position i (1-indexed) keeps its inc iff i in needed_set
        newval = {}
        cnt = 0
        for i in range(1, len(lst) + 1):
            if i in needed_set:
                cnt += 1
                newval[i] = cnt
        for i, (inst, u) in enumerate(lst, start=1):
            if i not in needed_set:
                si = inst.sync_info
                si.on_update = [x for x in si.on_update if x is not u]
        for w in wl:
            if w.wait_value and w.wait_value > 0:
                w.wait_value = newval[w.wait_value]


def _split_sync_waits(nc, maxw=1):
    """This walrus build accepts at most one sync wait per instruction.
    Hoist extra waits onto preceding NOPs on the same engine (engine
    execution is serial, so the semantics are identical)."""
    n_split = 0
    for fn in nc.m.functions:
        for blk in fn.blocks:
            insts = blk.instructions
            out = []
            for inst in insts:
                si = inst.sync_info
                waits = list(si.on_wait) if (si and si.on_wait) else []
                if len(waits) > maxw:
                    n_split += 1
                    extras = waits[:-maxw]
                    for i in range(0, len(extras), maxw):
                        nop = mybir.InstNoOp(
                            name=f"{inst.name}-ws{i}", hint="wsplit")
                        nop.engine = inst.engine
                        nop.sync_info = mybir.SyncInfo(
                            on_wait=extras[i:i + maxw], on_update=[])
                        out.append(nop)
                    si.on_wait = waits[-maxw:]
                out.append(inst)
            blk.instructions = out
    return n_split


def _build_tile(nc, tc, ext, vb_nonzero, b2_nonzero):
    x_ext, wq_ext, wk_ext, wv_ext, wo_ext = (
        ext["x_ext"], ext["wq_ext"], ext["wk_ext"], ext["wv_ext"], ext["wo_ext"])
    w1_ext, w2_ext = ext["w1_ext"], ext["w2_ext"]
    qb_ext, kb_ext, vb_ext, y1b_ext, b2_ext = (
        ext["qb_ext"], ext["kb_ext"], ext["vb_ext"], ext["y1b_ext"], ext["b2_ext"])
    mp1_ext, mp2_ext, out_ext = ext["mp1_ext"], ext["mp2_ext"], ext["out_ext"]

    Exp = mybir.ActivationFunctionType.Exp
    Silu = mybir.ActivationFunctionType.Silu
    Sqrt = mybir.ActivationFunctionType.Sqrt
    Ident = mybir.ActivationFunctionType.Identity
    Add = mybir.AluOpType.add
    Mult = mybir.AluOpType.mult
    Sub = mybir.AluOpType.subtract

    # One shared pool; tags are manually-assigned memory slots reused across
    # phases (Tile inserts WAR syncs on slot reuse). Sizes per partition:
    #   x32:   32KB   x (A..C)  -> vel fp8 (A) -> kT_bO fp8 (B odd) -> y1s (D)
    #   t16_1: 16KB   xnT (A)             -> ctxT (B..C)  -> y2T (D)
    #   t16_2: 16KB   qT (A..B)           -> ynT (C..D)
    #   t16_3: 16KB   kTl fp8 (A)         -> wo (C)       -> y2a (D)
    #   t16_4: 16KB   wk (A)              -> kslices0 fp8 (B even) -> w1h_a (D)
    #   t16_5: 16KB   wq (A)              -> kslices1 fp8 (B even) -> w1h_b (D)
    #   t16_6: 16.25  wv (A)              -> vts s0-7 fp8 (B)  -> w2h_a (D)
    #   t17:   16.25  (A)                 -> vts s8-15 fp8 (B) -> w2h_b (D)
    # r1 (fp32 residual after attention) is spilled to DRAM between C and D.
    with tc.tile_pool(name="mem", bufs=1) as memp, \
         tc.tile_pool(name="const", bufs=1) as constp, \
         tc.tile_pool(name="dram", bufs=1, space="DRAM") as dramp:
        ident = constp.tile([P, P], BF16)
        make_identity(nc, ident)
        eps_t = constp.tile([P, 1], F32)
        nc.vector.memset(eps_t, EPS)
        expoff = constp.tile([P, 1], F32)
        nc.vector.memset(expoff, EXP_OFF)

        # combined K+V collective buffers (fp8): chunk c covers batches
        # 2c,2c+1. Per partition: K part [bb2, j2, m8, q128] then V part
        # [mt4, 1040].
        KOF, VOF = 0, 4096
        CKV = 4096 + 4 * 1040
        ckv_in = [dramp.tile([P, CKV], FP8, name=f"ckv_in{c}")
                  for c in range(2)]
        ckv_out = [dramp.tile([NC, P, CKV], FP8, addr_space="Shared",
                              name=f"ckv_out{c}") for c in range(2)]
        r1d = dramp.tile([P, 8, D], F32)
        rdram = dramp

        # ================= Phase A: LN1, transpose, K/V (chunked AG), Q ====
        # x first on the sync queue (LN gates everything); weights on the
        # scalar queue; small consts on vector.
        x_sb = memp.tile([P, 8, D], F32, tag="x32", name="x_sb")
        for mt in range(4):
            nc.sync.dma_start(x_sb[:, mt, :], x_ext[mt])
        for mt in range(4, 8):
            nc.scalar.dma_start(x_sb[:, mt, :], x_ext[mt])
        xnT_sb = memp.tile([P, 8, D], BF16, tag="t16_1", name="xnT_sb")
        qT_sb = memp.tile([P, 8, D], BF16, tag="t16_2", name="qT_sb")
        kTl_sb = memp.tile([P, 4, 2, 8, P], FP8, tag="t16_3", name="kTl_sb")
        wk_sb = memp.tile([P, 8, 8, P], BF16, tag="t16_4", name="wk_sb")
        nc.gpsimd.dma_start(wk_sb[:], wk_ext[:])
        wq_sb = memp.tile([P, 8, 8, P], BF16, tag="t16_5", name="wq_sb")
        nc.gpsimd.dma_start(wq_sb[:], wq_ext[:])
        wv_sb = memp.tile([P, 8, D], BF16, tag="wv16", name="wv_sb")
        nc.gpsimd.dma_start(wv_sb[:], wv_ext[:])
        qb_sb = constp.tile([P, 8], F32)
        nc.gpsimd.dma_start(qb_sb[:], qb_ext[:])
        kb_sb = constp.tile([P, 8], F32)
        nc.gpsimd.dma_start(kb_sb[:], kb_ext[:])
        vb_sb = constp.tile([P, 8], F32)
        nc.gpsimd.dma_start(vb_sb[:], vb_ext[:])
        y1b_sb = constp.tile([P, 32], F32)
        nc.gpsimd.dma_start(y1b_sb[:], y1b_ext[:])
        b2bc_sb = constp.tile([P, D], F32)
        if b2_nonzero:
            nc.gpsimd.dma_start(b2bc_sb[:], b2_ext[:])
        mp1_sb = constp.tile([P, 8, P], BF16)
        nc.gpsimd.dma_start(mp1_sb[:], mp1_ext[:])
        mp2_sb = constp.tile([P, 8, P], BF16)
        nc.gpsimd.dma_start(mp2_sb[:], mp2_ext[:])

        with tc.tile_pool(name="ln", bufs=3) as lnp, \
             tc.tile_pool(name="psA", bufs=4, space="PSUM") as psA, \
             tc.tile_pool(name="psT", bufs=2, space="PSUM") as psT:
            def ln1_mt(mt):
                xv = x_sb[:, mt, :]
                stats = lnp.tile([P, 2, 6], F32, tag="stats", name="stats")
                nc.vector.bn_stats(stats[:, 0, :], xv[:, 0:512])
                nc.vector.bn_stats(stats[:, 1, :], xv[:, 512:1024])
                mv = lnp.tile([P, 2], F32, tag="mv", name="mv")
                nc.vector.bn_aggr(mv[:], stats[:])
                std = lnp.tile([P, 1], F32, tag="std", name="std")
                nc.scalar.activation(std[:], mv[:, 1:2], Sqrt, bias=eps_t[:])
                rstd = lnp.tile([P, 1], F32, tag="rstd", name="rstd")
                nc.vector.reciprocal(rstd[:], std[:])
                xn = lnp.tile([P, D], BF16, tag="xn", name="xn")
                nc.vector.tensor_scalar(
                    xn[:], xv, mv[:, 0:1], rstd[:], op0=Sub, op1=Mult)
                for g in range(2):
                    ps_t = psT.tile([P, 512], BF16, tag="pst", name="ps_t")
                    for k2 in range(4):
                        kt = g * 4 + k2
                        nc.tensor.transpose(
                            ps_t[:, k2 * P:(k2 + 1) * P],
                            xn[:, kt * P:(kt + 1) * P], ident[:])
                    nc.vector.tensor_copy(
                        xnT_sb[:, g * 4:(g + 1) * 4, mt * P:(mt + 1) * P],
                        ps_t[:].rearrange("p (a b) -> p a b", a=4))

            vel_sb = memp.tile([P, 8, 1040], FP8, tag="v8", name="vel_sb")

            # K, V per batch-pair chunk; one combined K+V AllGather per chunk
            # fires as soon as the chunk's inputs are in DRAM. Q last (keeps
            # PE busy during AGs).
            for n in range(2):
                for mt in range(4 * n, 4 * n + 4):
                    ln1_mt(mt)
                for m in range(8):
                    ps = psA.tile([P, 512], F32, tag="mm")
                    for kt in range(8):
                        nc.tensor.matmul(
                            ps[:], wk_sb[:, kt, m, :],
                            xnT_sb[:, kt, n * 512:(n + 1) * 512],
                            start=(kt == 0), stop=(kt == 7))
                    nc.scalar.activation(
                        kTl_sb[:, 2 * n:2 * n + 2, :, m, :],
                        ps[:].rearrange("p (a c t) -> p a c t", a=2, c=2),
                        Ident, bias=kb_sb[:, m:m + 1])
                    nc.scalar.dma_start(
                        bass.AP(tensor=ckv_in[n].tensor,
                                offset=ckv_in[n].offset + KOF + m * P,
                                ap=[[CKV, P], [2048, 2], [1024, 2], [1, P]]),
                        kTl_sb[:, 2 * n:2 * n + 2, :, m, :])
                for mt in range(4 * n, 4 * n + 4):
                    vv = vel_sb[:, mt, :].rearrange("p (h c) -> p h c", c=65)
                    nc.vector.memset(vv[:, :, 64:65], 1.0)
                    for nn in range(2):
                        ps = psA.tile([P, 512], F32, tag="mm")
                        for kt in range(8):
                            nc.tensor.matmul(
                                ps[:], xnT_sb[:, kt, mt * P:(mt + 1) * P],
                                wv_sb[:, kt, nn * 512:(nn + 1) * 512],
                                start=(kt == 0), stop=(kt == 7))
                        nc.vector.tensor_copy(
                            vv[:, 8 * nn:8 * nn + 8, 0:64],
                            ps[:].rearrange("p (h c) -> p h c", c=64))
                    l = mt - 4 * n
                    nc.gpsimd.dma_start(
                        ckv_in[n][:, VOF + l * 1040:VOF + (l + 1) * 1040],
                        vel_sb[:, mt, :])
                nc.gpsimd.collective_compute(
                    "AllGather", mybir.AluOpType.bypass,
                    replica_groups=[list(range(NC))],
                    ins=[ckv_in[n][:].opt()], outs=[ckv_out[n][:].opt()])

            for m in range(8):
                for n in range(2):
                    ps = psA.tile([P, 512], F32, tag="mm")
                    for kt in range(8):
                        nc.tensor.matmul(
                            ps[:], wq_sb[:, kt, m, :],
                            xnT_sb[:, kt, n * 512:(n + 1) * 512],
                            start=(kt == 0), stop=(kt == 7))
                    nc.scalar.activation(
                        qT_sb[:, m, n * 512:(n + 1) * 512], ps[:],
                        Ident, bias=qb_sb[:, m:m + 1])


        # ================= Phase B: attention + Wo/residual ================
        ctxT_sb = memp.tile([P, 8, D], BF16, tag="t16_1", name="ctxT_sb")
        wo_sb = memp.tile([P, 8, D], BF16, tag="t16_3", name="wo_sb")
        nc.scalar.dma_start(wo_sb[:], wo_ext[:])

        st2_by_mt = {}
        mv2_all = memp.tile([P, 8, 2], F32, tag="v8", name="mv2_all")
        rstd2_all = memp.tile([P, 8], F32, tag="v8b", name="rstd2_all")
        with tc.tile_pool(name="pt", bufs=3) as ptp, \
             tc.tile_pool(name="sm", bufs=2) as smp, \
             tc.tile_pool(name="stgB", bufs=3) as stgB, \
             tc.tile_pool(name="psS", bufs=2, space="PSUM") as psS, \
             tc.tile_pool(name="psC", bufs=2, space="PSUM") as psC:
            for b in range(B):
                c, bb = b // 2, b % 2
                ckv_o = ckv_out[c]
                # kT tiles are slot-major [P, slot, m, q] so the gather DMA
                # coalesces (m,q) and balances at 3 dims: [part, r, mq].
                if b % 2 == 0:
                    kT_b1 = memp.tile([P, 8, 8, P], FP8, tag="t16_4", name="kT_b1")
                    kT_b2 = memp.tile([P, 8, 8, P], FP8, tag="t16_5", name="kT_b2")
                    kdst = [kT_b1[:, :, :, :], kT_b2[:, :, :, :]]
                    kslices = [kT_b1, kT_b2]
                else:
                    kT_bO = memp.tile([P, 16, 8, P], FP8, tag="wv16", name="kT_bO")
                    kdst = [kT_bO[:, 0:8, :, :], kT_bO[:, 8:16, :, :]]
                    kslices = None
                for j in range(2):
                    src = bass.AP(
                        tensor=ckv_o.tensor,
                        offset=ckv_o.offset + KOF + bb * 2048 + j * 1024,
                        ap=[[CKV, P], [P * CKV, 8], [P, 8], [1, P]])
                    nc.sync.dma_start(kdst[j], src)
                vts = [memp.tile([P, 1040], FP8,
                                 tag=("t16_6" if s < 8 else "t17"),
                                 bufs=8, name=f"vt{s}") for s in range(16)]
                for s in range(16):
                    r, j = kv_src(s)
                    l = bb * 2 + j
                    nc.sync.dma_start(
                        vts[s][:],
                        ckv_o[r, :, VOF + l * 1040:VOF + (l + 1) * 1040])

                def kT_ap(pp_, m_, s_):
                    if kslices is not None:
                        return kslices[s_ // 8][pp_:pp_ + 64, s_ % 8, m_, :]
                    return kT_bO[pp_:pp_ + 64, s_, m_, :]

                def wo_sub(b_, idx):
                    # One (mt, n) chunk of Wo + residual for batch b_; fills
                    # PE gaps in the exp-bound attention stream. x_sb is
                    # still resident. LN2 stats ride on the live r1 tiles.
                    mt = 2 * b_ + idx // 2
                    n = idx % 2
                    ps = psC.tile([P, 512], F32, tag="ctx", name="wops")
                    for kt in range(8):
                        nc.tensor.matmul(
                            ps[:], ctxT_sb[:, kt, mt * P:(mt + 1) * P],
                            wo_sb[:, kt, n * 512:(n + 1) * 512],
                            start=(kt == 0), stop=(kt == 7))
                    stg = stgB.tile([P, 512], F32, tag="r1s", name="stg")
                    nc.vector.tensor_tensor(
                        stg[:], ps[:],
                        x_sb[:, mt, n * 512:(n + 1) * 512], Add)
                    if n == 0:
                        st2_by_mt[mt] = smp.tile(
                            [P, 2, 6], F32, tag="st2", bufs=2, name="st2")
                    nc.vector.bn_stats(st2_by_mt[mt][:, n, :], stg[:])
                    nc.gpsimd.dma_start(
                        r1d[:, mt, n * 512:(n + 1) * 512], stg[:])
                    if n == 1:
                        nc.vector.bn_aggr(mv2_all[:, mt, :], st2_by_mt[mt][:])
                        std2 = smp.tile([P, 1], F32, tag="std2", bufs=2,
                                        name="std2")
                        nc.scalar.activation(
                            std2[:], mv2_all[:, mt, 1:2], Sqrt, bias=eps_t[:])
                        nc.vector.reciprocal(
                            rstd2_all[:, mt:mt + 1], std2[:])

                stage = smp.tile([65, 16, 256], BF16, tag="stage", bufs=1)

                def pv_head(h, pT1, pT2):
                    ps_c = psC.tile([P, 256], F32, tag="ctx", name="ps_c")
                    for s in range(8):
                        nc.tensor.matmul(
                            ps_c[0:65, :],
                            vts[s][:, h * 65:h * 65 + 65],
                            pT1[:, s, :], start=(s == 0), stop=False,
                            skip_group_check=True)
                    for s in range(8):
                        nc.tensor.matmul(
                            ps_c[0:65, 128:256],
                            vts[8 + s][:, h * 65:h * 65 + 65],
                            pT2[:, s, :], start=False, stop=(s == 7),
                            skip_group_check=True)
                    # ctx (rows 0-63) + denominators (row 64), unscaled
                    nc.vector.tensor_copy(stage[:, h, :], ps_c[0:65, :])

                pend = None
                for hp in range(8):
                    # paired heads: h0 on PE row-group 0-63, h1 on 64-127 --
                    # their score matmuls run on disjoint sub-arrays. PV of
                    # the previous pair is interleaved between this pair's
                    # score chunks to keep both PE and ACT gapless.
                    hpair = (2 * hp, 2 * hp + 1)
                    m = hp
                    qa = {}
                    qb = {}
                    pTm = {}
                    for h in hpair:
                        pp = (h % 2) * 64
                        qa[h] = qT_sb[pp:pp + 64, m, b * 256:b * 256 + 256]
                        qb[h] = qT_sb[pp:pp + 64, m, b * 256 + 128:b * 256 + 256]
                        pTm[h] = ptp.tile([P, 3072], BF16, tag="pt", name=f"pTm{h}")
                    # chunk 0: qa x kv slots 0-5
                    sc0 = {}
                    for h in hpair:
                        sc0[h] = psS.tile([P, 1536], F32, tag="sc", name=f"sc0_{h}")
                    for s in range(6):
                        for h in hpair:
                            pp = (h % 2) * 64
                            nc.tensor.matmul(
                                sc0[h][:, s * 256:(s + 1) * 256],
                                kT_ap(pp, m, s), qa[h], start=True, stop=True)
                    if pend is not None:
                        pv_head(pend[0][0], pend[1][pend[0][0]], pend[2][pend[0][0]])
                    for h in hpair:
                        nc.scalar.activation(
                            pTm[h][:, 0:1536], sc0[h][:], Exp, bias=expoff[:])
                    # chunk 1: qa x kv slots 6-7, qb x kv slots 8-15
                    sc1 = {}
                    for h in hpair:
                        sc1[h] = psS.tile([P, 1536], F32, tag="sc", name=f"sc1_{h}")
                    for s in range(6, 8):
                        for h in hpair:
                            pp = (h % 2) * 64
                            nc.tensor.matmul(
                                sc1[h][:, (s - 6) * 256:(s - 5) * 256],
                                kT_ap(pp, m, s), qa[h], start=True, stop=True)
                    for s in range(8):
                        for h in hpair:
                            pp = (h % 2) * 64
                            nc.tensor.matmul(
                                sc1[h][:, 512 + s * P:512 + (s + 1) * P],
                                kT_ap(pp, m, 8 + s), qb[h], start=True, stop=True)
                    if pend is not None:
                        ph = pend[0][1]
                        pv_head(ph, pend[1][ph], pend[2][ph])
                    for h in hpair:
                        nc.scalar.activation(
                            pTm[h][:, 1536:3072], sc1[h][:], Exp, bias=expoff[:])

                    pT1 = {}
                    pT2 = {}
                    for h in hpair:
                        pT1[h] = pTm[h][:, 0:2048].rearrange(
                            "p (a b) -> p a b", b=256)
                        pT2[h] = pTm[h][:, 2048:3072].rearrange(
                            "p (a b) -> p a b", b=128)
                        nc.vector.tensor_tensor(
                            pT1[h][:, :, 0:P], pT1[h][:, :, 0:P],
                            mp1_sb[:], Mult)
                        nc.vector.tensor_tensor(
                            pT2[h][:], pT2[h][:], mp2_sb[:], Mult)
                    pend = (hpair, pT1, pT2)
                    if hp % 2 == 1 and b > 0:
                        wo_sub(b - 1, hp // 2)
                for h in pend[0]:
                    pv_head(h, pend[1][h], pend[2][h])

                # batched denominators: one reciprocal per batch
                den_d = rdram.tile([1, 16 * 256], BF16, tag="dend", bufs=2)
                nc.gpsimd.dma_start(
                    den_d[:],
                    stage[64:65, :, :].rearrange("p a b -> p (a b)"))
                den_sb = smp.tile([16, 256], BF16, tag="den")
                nc.gpsimd.dma_start(den_sb[:], bass.AP(
                    tensor=den_d.tensor, offset=den_d.offset,
                    ap=[[256, 16], [1, 256]]))
                rec_sb = smp.tile([16, 256], F32, tag="rec")
                nc.vector.reciprocal(rec_sb[:], den_sb[:])
                rec16_sb = smp.tile([16, 256], BF16, tag="rec16")
                nc.vector.tensor_copy(rec16_sb[:], rec_sb[:])
                rec_d = rdram.tile([16, 256], BF16, tag="recd", bufs=2)
                nc.gpsimd.dma_start(rec_d[:], rec16_sb[:])
                rec_b = smp.tile([64, 16, 256], BF16, tag="recb", bufs=1)
                nc.gpsimd.dma_start(rec_b[:], bass.AP(
                    tensor=rec_d.tensor, offset=rec_d.offset,
                    ap=[[0, 64], [256, 16], [1, 256]]))
                for h in range(H):
                    pp = (h % 2) * 64
                    m = h // 2
                    dst = ctxT_sb[pp:pp + 64, m, b * 256:b * 256 + 256]
                    nc.vector.tensor_tensor(
                        dst, stage[0:64, h, :], rec_b[:, h, :], Mult)
                    if vb_nonzero:
                        nc.vector.tensor_scalar_add(
                            dst, dst, vb_sb[pp:pp + 64, m:m + 1])
            for idx in range(4):
                wo_sub(3, idx)

        # ====== Phase C: LN2 normalize (stats precomputed) + transpose =====
        ynT_sb = memp.tile([P, 8, D], BF16, tag="t16_2", name="ynT_sb")

        with tc.tile_pool(name="ln2", bufs=3) as lnp, \
             tc.tile_pool(name="psT2", bufs=2, space="PSUM") as psT:
            for mt in range(8):
                rv = lnp.tile([P, D], F32, tag="r1v")
                nc.sync.dma_start(rv[:], r1d[:, mt, :])
                yn = lnp.tile([P, D], BF16, tag="yn")
                nc.vector.tensor_scalar(
                    yn[:], rv, mv2_all[:, mt, 0:1], rstd2_all[:, mt:mt + 1],
                    op0=Sub, op1=Mult)
                for g in range(2):
                    ps_t = psT.tile([P, 512], BF16, tag="pst")
                    for k2 in range(4):
                        kt = g * 4 + k2
                        nc.tensor.transpose(
                            ps_t[:, k2 * P:(k2 + 1) * P],
                            yn[:, kt * P:(kt + 1) * P], ident[:])
                    nc.vector.tensor_copy(
                        ynT_sb[:, g * 4:(g + 1) * 4, mt * P:(mt + 1) * P],
                        ps_t[:].rearrange("p (a b) -> p a b", a=4))

        # ====== Phase D: FFN (W2 in natural layout -> direct output) ======
        # W1 keeps the transposed layout (y1s = [fdimT, tok]); W2's rhs is
        # natural [f, dim] so its psum comes out [tok, dim] -- residual adds
        # and stores stream per (mt, n) with no final transposes.
        y2a_sb = memp.tile([P, 8, D], BF16, tag="t16_3", name="y2a_sb")

        with tc.tile_pool(name="stg", bufs=3) as stgp, \
             tc.tile_pool(name="psD", bufs=4, space="PSUM") as psA:
            for fh in range(2):
                w1h_a = memp.tile([P, 8, 8, P], BF16, tag="t16_4", name="w1h_a")
                nc.sync.dma_start(
                    w1h_a[:], w1_ext[:, :, fh * 16:fh * 16 + 8, :])
                w1h_b = memp.tile([P, 8, 8, P], BF16, tag="t16_5", name="w1h_b")
                nc.sync.dma_start(
                    w1h_b[:], w1_ext[:, :, fh * 16 + 8:fh * 16 + 16, :])
                w2n_a = memp.tile([P, 16, 512], BF16, tag="t16_1", name="w2n_a")
                nc.scalar.dma_start(
                    w2n_a[:], w2_ext[:, fh * 16:(fh + 1) * 16, 0:512])
                w2n_b = memp.tile([P, 16, 512], BF16, tag="wv16", name="w2n_b")
                nc.scalar.dma_start(
                    w2n_b[:], w2_ext[:, fh * 16:(fh + 1) * 16, 512:1024])
                y1s = memp.tile([P, 16, D], BF16, tag="x32", name="y1s")
                for mi in range(16):
                    w1t = (w1h_a if mi < 8 else w1h_b)
                    ps = psA.tile([P, 1024], F32, tag="mm", bufs=3)
                    for n in range(2):
                        for kt in range(8):
                            nc.tensor.matmul(
                                ps[:, n * 512:(n + 1) * 512],
                                w1t[:, kt, mi % 8, :],
                                ynT_sb[:, kt, n * 512:(n + 1) * 512],
                                start=(kt == 0), stop=(kt == 7))
                    nc.scalar.activation(
                        y1s[:, mi, :], ps[:],
                        Silu, bias=y1b_sb[:, fh * 16 + mi:fh * 16 + mi + 1])
                for mt in range(8):
                    for n in range(2):
                        w2n = w2n_a if n == 0 else w2n_b
                        ps = psA.tile([P, 512], F32, tag="mm2", bufs=2)
                        for kt in range(16):
                            nc.tensor.matmul(
                                ps[:], y1s[:, kt, mt * P:(mt + 1) * P],
                                w2n[:, kt, :],
                                start=(kt == 0), stop=(kt == 15))
                        if fh == 0:
                            if b2_nonzero:
                                nc.vector.tensor_tensor(
                                    y2a_sb[:, mt, n * 512:(n + 1) * 512],
                                    ps[:],
                                    b2bc_sb[:, n * 512:(n + 1) * 512], Add)
                            else:
                                nc.vector.tensor_copy(
                                    y2a_sb[:, mt, n * 512:(n + 1) * 512],
                                    ps[:])
                        else:
                            r1s = stgp.tile([P, 512], F32, tag="r1s")
                            nc.sync.dma_start(
                                r1s[:], r1d[:, mt, n * 512:(n + 1) * 512])
                            tmp = stgp.tile([P, 512], F32, tag="tmp")
                            nc.vector.tensor_tensor(
                                tmp[:], ps[:],
                                y2a_sb[:, mt, n * 512:(n + 1) * 512], Add)
                            stg = stgp.tile([P, 512], F32, tag="outs")
                            nc.vector.tensor_tensor(stg[:], tmp[:], r1s[:], Add)
                            nc.sync.dma_start(
                                out_ext[mt, :, n * 512:(n + 1) * 512], stg[:])


# ---------------------------------------------------------------------------
# host side
# ---------------------------------------------------------------------------

def _prep_inputs(hidden_state, attention_mask, Wq, Wk, Wv, Wo, ln1_g, ln1_b,
                 W1, b1, W2, b2, ln2_g, ln2_b):
    hs = np.asarray(hidden_state, np.float32)
    Wq = np.asarray(Wq, np.float32); Wk = np.asarray(Wk, np.float32)
    Wv = np.asarray(Wv, np.float32); Wo = np.asarray(Wo, np.float32)
    W1 = np.asarray(W1, np.float32); W2 = np.asarray(W2, np.float32)
    ln1_g = np.asarray(ln1_g, np.float32); ln1_b = np.asarray(ln1_b, np.float32)
    ln2_g = np.asarray(ln2_g, np.float32); ln2_b = np.asarray(ln2_b, np.float32)
    b1 = np.asarray(b1, np.float32); b2 = np.asarray(b2, np.float32)
    am = np.asarray(attention_mask)

    Wq_e = (ln1_g[:, None] * Wq) / SCALE
    Wk_e = ln1_g[:, None] * Wk
    Wv_e = ln1_g[:, None] * Wv
    W1_e = ln2_g[:, None] * W1
    qb = (ln1_b @ Wq) / SCALE
    kb = ln1_b @ Wk
    vb = ln1_b @ Wv
    y1b = ln2_b @ W1 + b1

    def lhst_tiles(w, kt, m):  # [K, M] -> [128, kt, m, 128]
        return np.ascontiguousarray(
            w.reshape(kt, P, m, P).transpose(1, 0, 2, 3)).astype(NPBF16)

    def rhs_tiles(w, kt):      # [K, N] -> [128, kt, N]
        return np.ascontiguousarray(
            w.reshape(kt, P, -1).transpose(1, 0, 2)).astype(NPBF16)

    def pvec(v):               # [D] -> [128, D//128] per-partition layout
        return np.ascontiguousarray(v.reshape(-1, P).T).astype(np.float32)

    common = {
        "wq": lhst_tiles(Wq_e, 8, 8), "wk": lhst_tiles(Wk_e, 8, 8),
        "wv": rhs_tiles(Wv_e, 8), "wo": rhs_tiles(Wo, 8),
        "w1": lhst_tiles(W1_e, 8, 32), "w2": rhs_tiles(W2, 32),
        "qb": pvec(qb), "kb": pvec(kb), "vb": pvec(vb),
        "y1b": pvec(y1b),
        "b2t": np.ascontiguousarray(np.tile(b2[None, :], (P, 1))).astype(np.float32),
    }

    kk = np.arange(P)[:, None]
    qq = np.arange(P)[None, :]
    tri = (kk <= qq)  # [128,128] lower-tri in (k_partition, q_free)

    in_maps = []
    for i in range(NC):
        blkA, blkB = i, 15 - i
        x_i = np.empty((8, P, D), np.float32)
        for b in range(B):
            x_i[b * 2 + 0] = hs[b, blkA * P:(blkA + 1) * P]
            x_i[b * 2 + 1] = hs[b, blkB * P:(blkB + 1) * P]
        mp1 = np.zeros((P, 8, P), np.float32)
        mp2 = np.zeros((P, 8, P), np.float32)
        for s in range(8):
            if s < blkA:
                mp1[:, s, :] = 1.0
            elif s == blkA:
                mp1[:, s, :] = tri
        for t2 in range(8):
            g = 15 - t2   # slot 8+t2 holds seq block 15-t2 (rank t2's j=1)
            if g < blkB:
                mp2[:, t2, :] = 1.0
            elif g == blkB:
                mp2[:, t2, :] = tri
        m = dict(common)
        m["x"] = x_i
        m["mp1"] = mp1.astype(NPBF16)
        m["mp2"] = mp2.astype(NPBF16)
        in_maps.append(m)

    vb_nonzero = not np.allclose(vb, 0.0)
    b2_nonzero = not np.allclose(b2, 0.0)
    return in_maps, vb_nonzero, b2_nonzero


def run(inputs, trace=False):
    in_maps, vb_nonzero, b2_nonzero = _prep_inputs(**inputs)
    nc = build_graph(vb_nonzero, b2_nonzero)
    res = run_bass_kernel_spmd(nc, in_maps, list(range(NC)), trace=trace)
    outs = res.results
    out_full = np.empty((B, S, D), np.float32)
    for i in range(NC):
        o = np.asarray(outs[i]["out"])
        for b in range(B):
            out_full[b, i * P:(i + 1) * P] = o[b * 2 + 0]
            out_full[b, (15 - i) * P:(16 - i) * P] = o[b * 2 + 1]
    return out_full, res


def kernel(**inputs):
    out, _ = run(inputs, trace=False)
    return out
